# revision 24
# baseline (speedup 1.0000x reference)
"""Trainium2 Bass kernel for nn_DiffNet (gnn_message_passing).

The reference's per-element "edge MLP" over the meta stack
(vi, W, vj) -> two 1x1 convs -> weighted sum over the input dim is
linear in its 3 channels, so it collapses algebraically.  With
g = conv1_w.T @ conv2_w[0]  (3 scalars), hb = conv1_b@conv2_w[0]+conv2_b[0],
z = vi @ W.T (no bias), s1[b] = sum_i vi[b,i], s2[b] = sum_i vi[b,i]^2:

    out[b,o] = relu(z+b)[b,o] * (1 + scale*g2*s1[b])
             + scale*(g0*s2[b] + g1*z[b,o] + hb*s1[b])

so the whole network is 3 small matmuls + elementwise, and the problem
is memory-bound on the fc weights.

v3 (vs the 32-36us f32r baseline, which was bound by a single HWDGE
queue streaming 3.4 MB of fp32 at ~158 GB/s with double-pass LOW_HIGH
matmuls):
  * everything on the matmul dataflow is fp16 -> 1.6 MB of wire
    traffic and single-pass PE matmuls (tolerance is 2e-2; measured
    baseline err 1.1e-3, fp16 adds ~1e-3).
  * DMAs are split half/half between the qSync and qScalar HWDGE
    rings so the 16 shared SDMA engines are fed from two descriptor
    rings; x is its own small tensor loaded FIRST on both rings (in
    v2 it rode the slow SWDGE ring and landed at 14us, idling every
    engine); the biases+eye ride the tail of the w1 stream.
  * tiny constants (ones column for the K-dir sums, the alpha/beta
    coefficient matrices) are memset on-chip instead of DMAed.
  * the PE HAM clock gate defaults to 1.2 GHz and only ramps to
    2.4 GHz after ~3.4us of sustained activity (v2's z matmuls all
    ran at cold-clock rate, 427ns per 512-wide chunk).  A run of
    dummy matmuls on memset scratch keeps the PE busy from ~1us
    while the weights stream in, so the real z matmuls run warm.

Distribution (8 cores, no collectives): fc1/fc2 replicated, fc3
sharded over its output dim (32 cols/core); full batch B=32 on every
core; host concatenates the 8 [32,32] output shards.

On-core layout: activations live transposed [feature(partitions),
batch] in 128-row chunks; weights are passed pre-transposed [in, out].
Matmuls put the tiny activation tile stationary and stream the weight
chunk [128, 512] as the moving operand.  The z output lands
[batch, out] in fp32 PSUM; a PE transpose brings each 128-col chunk
back to [out, batch] where relu-bias, the k1*z term and the per-batch
alpha/beta scalars (broadcast across partitions via a rank-1 ones
matmul) are applied with a few wide DVE/ACT ops.
"""

import sys

if "/opt/trn_rl_repo" not in sys.path:
    sys.path.insert(0, "/opt/trn_rl_repo")

import numpy as np


def _install_ntff_hook_shim():
    """This image's antenv lacks ``axon_hooks``; bass_utils hard-imports it
    when tracing under axon.  Provide the module and register the ctypes
    NTFF hook from trn_agent_boot so ``trace=True`` yields exec_time_ns."""
    import types

    if "antenv.axon_hooks" in sys.modules:
        return
    try:
        import antenv

        mod = types.ModuleType("antenv.axon_hooks")
        _h = [None]
        mod.set_axon_ntff_profile_hook = lambda hook: _h.__setitem__(0, hook)
        mod.get_axon_ntff_profile_hook = lambda: _h[0]
        sys.modules["antenv.axon_hooks"] = mod
        antenv.axon_hooks = mod
        from trn_agent_boot.trn_boot import _ntff_profile_via_ctypes

        mod.set_axon_ntff_profile_hook(
            _ntff_profile_via_ctypes("/opt/axon/libaxon_pjrt.so")
        )
    except Exception:
        pass


_install_ntff_hook_shim()

N_CORES = 8
B = 32
I1, O1, O2, O3 = 1024, 512, 512, 256
O3L = O3 // N_CORES  # fc3 output cols per core
RATE = 0.1

W1W = 8 * O1  # w1 DRAM cols: the 8 K-chunks
W2W = 4 * O2 + 4 * O3L  # w2 DRAM tensor cols: w2 | w3
# bias rows tensor [1, BRW] f16: fc1_b | fc2_b | fc3_b shard (these feed
# K=1 rank-1 matmuls that fold each bias into its z accumulation, and a
# matmul operand must sit at base partition 0 -> ship as rows directly)
BRW = O1 + O2 + 128

_CACHE = {}
LAST_RESULTS = None  # BassKernelResults of the most recent run (for test.py)


def _build(k0, k1, k2, kb):
    import concourse.bacc as bacc
    import concourse.mybir as mybir
    import concourse.tile as tile
    import concourse.bass as bass
    from concourse.masks import make_identity

    f32 = mybir.dt.float32
    f16 = mybir.dt.float16
    AF = mybir.ActivationFunctionType
    ALU = mybir.AluOpType

    nc = bacc.Bacc(
        "TRN2", target_bir_lowering=False, debug=False, num_devices=N_CORES
    )

    xt = nc.declare_dram_parameter("xt", [128, 8 * B], f16, isOutput=False)
    w1 = nc.declare_dram_parameter("w1t", [128, W1W], f16, isOutput=False)
    w2 = nc.declare_dram_parameter("w2t", [128, W2W], f16, isOutput=False)
    br = nc.declare_dram_parameter("brt", [1, BRW], f16, isOutput=False)
    out_d = nc.declare_dram_parameter("out", [B, O3L], f32, isOutput=True)

    with tile.TileContext(nc) as tc:
        with (
            tc.tile_pool(name="wts", bufs=1) as wp,
            tc.tile_pool(name="act", bufs=1) as ap,
            tc.tile_pool(name="ps", bufs=1, space=bass.MemorySpace.PSUM) as pp,
        ):
            tx = wp.tile([128, 8 * B], f16, tag="x")
            tw1 = wp.tile([128, W1W], f16, tag="w1")
            tw2 = wp.tile([128, W2W], f16, tag="w2")
            tw3 = tw2[:, 4 * O2 : 4 * O2 + 4 * O3L]

            tbr = wp.tile([1, BRW], f16, tag="brow")
            t1k = wp.tile([128, 1], f16, tag="ones")
            t1r = wp.tile([1, B], f16, tag="onesr")
            tka = wp.tile([96, 128], f16, tag="ka")
            tkb = wp.tile([96, 128], f16, tag="kb")
            teye = wp.tile([128, 128], f16, tag="eye")
            tscr = wp.tile([128, O1], f16, tag="scr")  # PE warm-up fodder

            # -- DMAs, split between the two HWDGE rings.  x whole on
            # qSync first, then the one-descriptor bias-rows tensor; w1
            # in 4 chunks, 2 per ring (z1 accumulates in arrival
            # order); w3 rides the tail of w2's scalar half.
            nc.sync.dma_start(tx[:], xt[:])
            nc.sync.dma_start(tbr[:], br[:])
            nc.sync.dma_start(tw1[:, 0 : 2 * O1], w1[:, 0 : 2 * O1])
            nc.scalar.dma_start(tw1[:, 4 * O1 : 6 * O1], w1[:, 4 * O1 : 6 * O1])
            nc.sync.dma_start(tw1[:, 2 * O1 : 4 * O1], w1[:, 2 * O1 : 4 * O1])
            nc.scalar.dma_start(tw1[:, 6 * O1 : 8 * O1], w1[:, 6 * O1 : 8 * O1])
            nc.sync.dma_start(tw2[:, 0 : 2 * O2], w2[:, 0 : 2 * O2])
            nc.scalar.dma_start(tw2[:, 2 * O2 : W2W], w2[:, 2 * O2 : W2W])

            # PE warm-up: the HAM clock gate only ramps 1.2 -> 2.4 GHz
            # after ~3.4us of sustained PE activity.  The scratch is
            # memset on the otherwise-idle GpSimd so the warm-up only
            # waits on that one op and starts the moment the PE queue
            # opens; the real matmuls then run warm.  (warm_ps shares
            # the "z" PSUM tag, so z1 orders after it.)
            nc.gpsimd.memset(tscr[:], 0.0)
            warm_ps = pp.tile([1, O1], f32, tag="z")
            for _ in range(7):
                nc.tensor.matmul(
                    warm_ps[:], tscr[:, 0:1], tscr[:], start=True, stop=True
                )

            nc.vector.memset(t1k[:], 1.0)
            nc.vector.memset(t1r[:], 1.0)
            nc.vector.memset(tka[:], 0.0)
            nc.vector.memset(tkb[:], 0.0)
            # alpha = k2*s1 + 1 ; beta = kb*s1 + k0*s2
            # (s_sb rows: 0 = s1, 32 = s2, 64 = ones)
            nc.vector.memset(tka[0:1, :], k2)
            nc.vector.memset(tka[64:65, :], 1.0)
            nc.vector.memset(tkb[0:1, :], kb)
            nc.vector.memset(tkb[32:33, :], k0)
            # identity for the PE transposes, built on the idle GpSimd
            make_identity(nc, teye[:])

            def stats_row(a_tile, n_c, n_oc, tag):
                """alpha/beta [128, n_oc*B] f32 for the [out, batch]
                tails: rows all equal, the [128,B] per-batch scalars
                replicated n_oc times for one full-width DVE apply."""
                asq = ap.tile([128, n_c * B], f16, tag=tag + "sq")
                nc.vector.tensor_tensor(asq[:], a_tile, a_tile, ALU.mult)
                s1_ps = pp.tile([1, B], f32, tag="s1")
                s2_ps = pp.tile([1, B], f32, tag="s2")
                for c in range(n_c):
                    nc.tensor.matmul(
                        s1_ps[:], t1k[:], a_tile[:, c * B : (c + 1) * B],
                        start=(c == 0), stop=(c == n_c - 1),
                    )
                for c in range(n_c):
                    nc.tensor.matmul(
                        s2_ps[:], t1k[:], asq[:, c * B : (c + 1) * B],
                        start=(c == 0), stop=(c == n_c - 1),
                    )
                # engine writes must start at partition 0/32/64 -> spread
                # (s1, s2, 1) over those rows; memset first so junk
                # partitions are finite (their K coefficients are 0) and
                # row 64 is the ones row
                s_sb = ap.tile([96, B], f16, tag=tag + "row")
                nc.vector.memset(s_sb[:], 1.0)
                nc.scalar.copy(s_sb[0:1, :], s1_ps[:])
                nc.scalar.copy(s_sb[32:33, :], s2_ps[:])
                ab_ps = pp.tile([128, 2 * n_oc * B], f32, tag="ab")
                for oc in range(n_oc):
                    nc.tensor.matmul(
                        ab_ps[:, oc * B : (oc + 1) * B],
                        tka[:], s_sb[:], start=True, stop=True,
                    )
                    nc.tensor.matmul(
                        ab_ps[:, (n_oc + oc) * B : (n_oc + oc + 1) * B],
                        tkb[:], s_sb[:], start=True, stop=True,
                    )
                ab_sb = ap.tile([128, 2 * n_oc * B], f32, tag=tag + "sb")
                nc.vector.tensor_scalar_add(ab_sb[:], ab_ps[:], 0.0)
                return ab_sb[:, 0 : n_oc * B], ab_sb[:, n_oc * B :]

            def z_mms(a_tile, w_tile, ics, ow, boff, li):
                """z_ps [B, ow] = a.T @ w + bias row; `ics` gives the
                accumulation order (matched to DMA arrival order); the
                bias lands via K=1 rank-1 matmuls closing the group."""
                z_ps = pp.tile([B, ow], f32, tag="z")
                for j, ic in enumerate(ics):
                    nc.tensor.matmul(
                        z_ps[:],
                        a_tile[:, ic * B : (ic + 1) * B],
                        w_tile[:, ic * ow : (ic + 1) * ow],
                        start=(j == 0), stop=False,
                    )
                for g in range(ow // 128):
                    nc.tensor.matmul(
                        z_ps[:, g * 128 : (g + 1) * 128],
                        t1r[:],
                        tbr[0:1, boff + g * 128 : boff + (g + 1) * 128],
                        start=False, stop=(g == ow // 128 - 1),
                    )
                return z_ps

            def tail_row(z_ps, n_oc, alpha, beta, out_view, li):
                """[B, ow] z -> transpose to [out, batch], relu (bias
                already folded into z), combine full-width."""
                ow = n_oc * 128
                z_sb = ap.tile([B, ow], f16, tag=f"zsb{li}")
                nc.scalar.copy(z_sb[:], z_ps[:])
                zt_ps = pp.tile([128, n_oc * B], f16, tag="zt")
                for oc in range(n_oc):
                    nc.tensor.transpose(
                        zt_ps[:, oc * B : (oc + 1) * B],
                        z_sb[:, oc * 128 : (oc + 1) * 128],
                        teye[0:B, 0:B],
                    )
                vjt = ap.tile([128, n_oc * B], f32, tag=f"vj{li}")
                t_sb = ap.tile([128, n_oc * B], f32, tag=f"t{li}")
                nc.scalar.activation(vjt[:], zt_ps[:], AF.Relu)
                nc.vector.scalar_tensor_tensor(
                    t_sb[:], zt_ps[:], k1, beta, ALU.mult, ALU.add
                )
                nc.vector.tensor_tensor(vjt[:], vjt[:], alpha, ALU.mult)
                nc.vector.tensor_tensor(out_view[:], vjt[:], t_sb[:], ALU.add)

            # ---- layer 1: stats fill the PE while fc1 streams in
            al1, be1 = stats_row(tx, 8, 4, "ab1")
            z1 = z_mms(tx, tw1, [4, 5, 0, 1, 6, 7, 2, 3], O1, 0, 1)
            a2 = ap.tile([128, 4 * B], f16, tag="a2")
            tail_row(z1, 4, al1, be1, a2[:], 1)

            # ---- layer 2: stats before z so alpha/beta are ready when
            # the tail needs them
            al2, be2 = stats_row(a2[:], 4, 4, "ab2")
            z2 = z_mms(a2[:], tw2, [2, 3, 0, 1], O2, O1, 2)
            a3 = ap.tile([128, 4 * B], f16, tag="a3")
            tail_row(z2, 4, al2, be2, a3[:], 2)

            # ---- layer 3 stays in [batch, out]: stats as columns via
            # aT@ones matmuls, alpha/beta become per-partition scalars,
            # no transpose / broadcast / s_sb hop on the end path
            asq3 = ap.tile([128, 4 * B], f16, tag="a3sq")
            nc.vector.tensor_tensor(asq3[:], a3[:], a3[:], ALU.mult)
            s1c = pp.tile([B, 1], f32, tag="s1c")
            s2c = pp.tile([B, 1], f32, tag="s2c")
            for c in range(4):
                nc.tensor.matmul(
                    s1c[:], a3[:, c * B : (c + 1) * B], t1k[:],
                    start=(c == 0), stop=(c == 3),
                )
            for c in range(4):
                nc.tensor.matmul(
                    s2c[:], asq3[:, c * B : (c + 1) * B], t1k[:],
                    start=(c == 0), stop=(c == 3),
                )
            abc = ap.tile([B, 2], f32, tag="abc")
            alpha3, beta3 = abc[:, 0:1], abc[:, 1:2]
            nc.vector.tensor_scalar(alpha3, s1c[:], k2, 1.0, ALU.mult, ALU.add)
            nc.vector.tensor_scalar(beta3, s2c[:], k0, None, ALU.mult)
            nc.vector.scalar_tensor_tensor(
                beta3, s1c[:], kb, beta3, ALU.mult, ALU.add
            )
            z3_ps = pp.tile([B, O3L], f32, tag="z")
            for j, ic in enumerate([0, 1, 2, 3]):
                nc.tensor.matmul(
                    z3_ps[:],
                    a3[:, ic * B : (ic + 1) * B],
                    tw3[:, ic * O3L : (ic + 1) * O3L],
                    start=(j == 0), stop=False,
                )
            nc.tensor.matmul(
                z3_ps[:], t1r[:],
                tbr[0:1, O1 + O2 : O1 + O2 + O3L],
                start=False, stop=True,
            )
            vj3 = ap.tile([B, O3L], f32, tag="vj3")
            t3 = ap.tile([B, O3L], f32, tag="t3")
            out_sb = ap.tile([B, O3L], f32, tag="o3")
            nc.scalar.activation(vj3[:], z3_ps[:], AF.Relu)
            nc.vector.tensor_scalar(t3[:], z3_ps[:], k1, beta3, ALU.mult, ALU.add)
            nc.vector.scalar_tensor_tensor(
                out_sb[:], vj3[:], alpha3, t3[:], ALU.mult, ALU.add
            )

            nc.sync.dma_start(out_d[:], out_sb[:], single_packet=True)

    nc.compile()
    return nc


def kernel(**inputs):
    from concourse.bass_utils import run_bass_kernel_spmd

    x = np.ascontiguousarray(np.asarray(inputs["x"], dtype=np.float32))
    fc1_w = np.asarray(inputs["fc1_w"], dtype=np.float32)
    fc1_b = np.asarray(inputs["fc1_b"], dtype=np.float32)
    fc2_w = np.asarray(inputs["fc2_w"], dtype=np.float32)
    fc2_b = np.asarray(inputs["fc2_b"], dtype=np.float32)
    fc3_w = np.asarray(inputs["fc3_w"], dtype=np.float32)
    fc3_b = np.asarray(inputs["fc3_b"], dtype=np.float32)
    c1w = np.asarray(inputs["conv1_w"], dtype=np.float32)
    c1b = np.asarray(inputs["conv1_b"], dtype=np.float32)
    c2w = np.asarray(inputs["conv2_w"], dtype=np.float32)
    c2b = np.asarray(inputs["conv2_b"], dtype=np.float32)
    bn = float(np.asarray(inputs["batch_num"]).astype(np.float64))

    scale = np.float32(RATE) / np.float32(bn)
    g = (c1w.T @ c2w[0]).astype(np.float32)  # [3]
    hb = np.float32(c1b @ c2w[0] + c2b[0])
    k0 = float(scale * g[0])
    k1 = float(scale * g[1])
    k2 = float(scale * g[2])
    kb = float(scale * hb)

    key = (k0, k1, k2, kb)
    if key not in _CACHE:
        _CACHE[key] = _build(*key)
    nc = _CACHE[key]

    def pack(m, n_c, width):  # [n_c*128, width] -> [128, n_c*width]
        return np.ascontiguousarray(
            m.reshape(n_c, 128, width).transpose(1, 0, 2).reshape(128, n_c * width)
        )

    xt_h = pack(x.T, 8, B).astype(np.float16)
    w1_h = pack(fc1_w.T, 8, O1).astype(np.float16)
    w2_base = pack(fc2_w.T, 4, O2).astype(np.float16)

    in_maps = []
    for c in range(N_CORES):
        br_h = np.zeros((1, BRW), dtype=np.float16)
        br_h[0, 0:O1] = fc1_b
        br_h[0, O1 : O1 + O2] = fc2_b
        br_h[0, O1 + O2 : O1 + O2 + O3L] = fc3_b[c * O3L : (c + 1) * O3L]
        w2c = np.zeros((128, W2W), dtype=np.float16)
        w2c[:, 0 : 4 * O2] = w2_base
        w2c[:, 4 * O2 :] = pack(
            fc3_w[c * O3L : (c + 1) * O3L].T, 4, O3L
        ).astype(np.float16)
        in_maps.append(dict(xt=xt_h, w1t=w1_h, w2t=w2c, brt=br_h))

    res = run_bass_kernel_spmd(nc, in_maps, list(range(N_CORES)))
    global LAST_RESULTS
    LAST_RESULTS = res
    return np.ascontiguousarray(
        np.concatenate([res.results[c]["out"] for c in range(N_CORES)], axis=1)
    ).astype(np.float32)


if __name__ == "__main__":
    rng = np.random.default_rng(0)

    def lin(fo, fi):
        bound = 1.0 / np.sqrt(fi)
        return (
            rng.uniform(-bound, bound, (fo, fi)).astype(np.float32),
            rng.uniform(-bound, bound, (fo,)).astype(np.float32),
        )

    fc1_w, fc1_b = lin(512, 1024)
    fc2_w, fc2_b = lin(512, 512)
    fc3_w, fc3_b = lin(256, 512)
    c1w, c1b = lin(8, 3)
    c2w, c2b = lin(1, 8)
    ins = dict(
        x=rng.standard_normal((32, 1024)).astype(np.float32),
        fc1_w=fc1_w, fc1_b=fc1_b, fc2_w=fc2_w, fc2_b=fc2_b,
        fc3_w=fc3_w, fc3_b=fc3_b,
        conv1_w=c1w, conv1_b=c1b, conv2_w=c2w, conv2_b=c2b,
        batch_num=10,
    )
    out = kernel(**ins)
    print("kernel out", out.shape, out.dtype, float(np.abs(out).max()))


# revision 31
# speedup vs baseline: 1.0169x; 1.0169x over previous
"""Trainium2 Bass kernel for nn_DiffNet (gnn_message_passing).

The reference's per-element "edge MLP" over the meta stack
(vi, W, vj) -> two 1x1 convs -> weighted sum over the input dim is
linear in its 3 channels, so it collapses algebraically.  With
g = conv1_w.T @ conv2_w[0]  (3 scalars), hb = conv1_b@conv2_w[0]+conv2_b[0],
z = vi @ W.T (no bias), s1[b] = sum_i vi[b,i], s2[b] = sum_i vi[b,i]^2:

    out[b,o] = relu(z+b)[b,o] * (1 + scale*g2*s1[b])
             + scale*(g0*s2[b] + g1*z[b,o] + hb*s1[b])

so the whole network is 3 small matmuls + elementwise, and the problem
is memory-bound on the fc weights.

v3 (vs the 32-36us f32r baseline, which was bound by a single HWDGE
queue streaming 3.4 MB of fp32 at ~158 GB/s with double-pass LOW_HIGH
matmuls):
  * everything on the matmul dataflow is fp16 -> 1.6 MB of wire
    traffic and single-pass PE matmuls (tolerance is 2e-2; measured
    baseline err 1.1e-3, fp16 adds ~1e-3).
  * DMAs are split half/half between the qSync and qScalar HWDGE
    rings so the 16 shared SDMA engines are fed from two descriptor
    rings; x is its own small tensor loaded FIRST on both rings (in
    v2 it rode the slow SWDGE ring and landed at 14us, idling every
    engine); the biases+eye ride the tail of the w1 stream.
  * tiny constants (ones column for the K-dir sums, the alpha/beta
    coefficient matrices) are memset on-chip instead of DMAed.
  * the PE HAM clock gate defaults to 1.2 GHz and only ramps to
    2.4 GHz after ~3.4us of sustained activity (v2's z matmuls all
    ran at cold-clock rate, 427ns per 512-wide chunk).  A run of
    dummy matmuls on memset scratch keeps the PE busy from ~1us
    while the weights stream in, so the real z matmuls run warm.

Distribution (8 cores, no collectives): fc1/fc2 replicated, fc3
sharded over its output dim (32 cols/core); full batch B=32 on every
core; host concatenates the 8 [32,32] output shards.

On-core layout: activations live transposed [feature(partitions),
batch] in 128-row chunks; weights are passed pre-transposed [in, out].
Matmuls put the tiny activation tile stationary and stream the weight
chunk [128, 512] as the moving operand.  The z output lands
[batch, out] in fp32 PSUM; a PE transpose brings each 128-col chunk
back to [out, batch] where relu-bias, the k1*z term and the per-batch
alpha/beta scalars (broadcast across partitions via a rank-1 ones
matmul) are applied with a few wide DVE/ACT ops.
"""

import sys

if "/opt/trn_rl_repo" not in sys.path:
    sys.path.insert(0, "/opt/trn_rl_repo")

import numpy as np


def _install_ntff_hook_shim():
    """This image's antenv lacks ``axon_hooks``; bass_utils hard-imports it
    when tracing under axon.  Provide the module and register the ctypes
    NTFF hook from trn_agent_boot so ``trace=True`` yields exec_time_ns."""
    import types

    if "antenv.axon_hooks" in sys.modules:
        return
    try:
        import antenv

        mod = types.ModuleType("antenv.axon_hooks")
        _h = [None]
        mod.set_axon_ntff_profile_hook = lambda hook: _h.__setitem__(0, hook)
        mod.get_axon_ntff_profile_hook = lambda: _h[0]
        sys.modules["antenv.axon_hooks"] = mod
        antenv.axon_hooks = mod
        from trn_agent_boot.trn_boot import _ntff_profile_via_ctypes

        mod.set_axon_ntff_profile_hook(
            _ntff_profile_via_ctypes("/opt/axon/libaxon_pjrt.so")
        )
    except Exception:
        pass


_install_ntff_hook_shim()

N_CORES = 8
B = 32
I1, O1, O2, O3 = 1024, 512, 512, 256
O3L = O3 // N_CORES  # fc3 output cols per core
RATE = 0.1

W1W = 8 * O1  # w1 DRAM cols: the 8 K-chunks
W2W = 4 * O2 + 4 * O3L  # w2 DRAM tensor cols: w2 | w3
# bias rows tensor [1, BRW] f16: fc1_b | fc2_b | fc3_b shard (these feed
# K=1 rank-1 matmuls that fold each bias into its z accumulation, and a
# matmul operand must sit at base partition 0 -> ship as rows directly)
BRW = O1 + O2 + 128

_CACHE = {}
LAST_RESULTS = None  # BassKernelResults of the most recent run (for test.py)


def _build(k0, k1, k2, kb):
    import concourse.bacc as bacc
    import concourse.mybir as mybir
    import concourse.tile as tile
    import concourse.bass as bass
    from concourse.masks import make_identity

    f32 = mybir.dt.float32
    f16 = mybir.dt.float16
    AF = mybir.ActivationFunctionType
    ALU = mybir.AluOpType

    nc = bacc.Bacc(
        "TRN2", target_bir_lowering=False, debug=False, num_devices=N_CORES
    )

    xt = nc.declare_dram_parameter("xt", [128, 8 * B], f16, isOutput=False)
    w1 = nc.declare_dram_parameter("w1t", [128, W1W], f16, isOutput=False)
    w2 = nc.declare_dram_parameter("w2t", [128, W2W], f16, isOutput=False)
    br = nc.declare_dram_parameter("brt", [1, BRW], f16, isOutput=False)
    out_d = nc.declare_dram_parameter("out", [B, O3L], f32, isOutput=True)

    with tile.TileContext(nc) as tc:
        with (
            tc.tile_pool(name="wts", bufs=1) as wp,
            tc.tile_pool(name="act", bufs=1) as ap,
            tc.tile_pool(name="ps", bufs=1, space=bass.MemorySpace.PSUM) as pp,
        ):
            tx = wp.tile([128, 8 * B], f16, tag="x")
            tw1 = wp.tile([128, W1W], f16, tag="w1")
            tw2 = wp.tile([128, W2W], f16, tag="w2")
            tw3 = tw2[:, 4 * O2 : 4 * O2 + 4 * O3L]
            tbr = wp.tile([1, BRW], f16, tag="brow")
            t1k = wp.tile([128, 1], f16, tag="ones")
            t1r = wp.tile([1, B], f16, tag="onesr")
            teye = wp.tile([B, B], f16, tag="eye")
            tscr = wp.tile([128, O1], f16, tag="scr")  # PE warm-up fodder

            # -- DMAs, split between the two HWDGE rings in few big
            # pieces (per-ring throughput is descriptor-rate-bound, so
            # bigger per-partition lines move more bytes per slot).
            # x + bias rows first on qSync; z accumulation consumes the
            # w chunks in arrival order.
            nc.sync.dma_start(tx[:], xt[:])
            nc.scalar.dma_start(tbr[:], br[:])
            nc.sync.dma_start(tw1[:, 0 : 4 * O1], w1[:, 0 : 4 * O1])
            nc.scalar.dma_start(tw1[:, 4 * O1 : 8 * O1], w1[:, 4 * O1 : 8 * O1])
            nc.sync.dma_start(tw2[:, 0 : 2 * O2], w2[:, 0 : 2 * O2])
            nc.scalar.dma_start(tw2[:, 2 * O2 : W2W], w2[:, 2 * O2 : W2W])

            # PE warm-up: the HAM clock gate only ramps 1.2 -> 2.4 GHz
            # after ~3.4us of sustained PE activity.  The scratch is
            # memset on the otherwise-idle GpSimd so the warm-up starts
            # the moment the PE queue opens; the real matmuls then run
            # warm.  (warm_ps shares the "z1" PSUM tag, so z1 orders
            # after it.)
            nc.gpsimd.memset(tscr[:], 0.0)
            warm_ps = pp.tile([128, O1], f32, tag="z1")
            for _ in range(7):
                # full 128x128 stationary: the HAM watches array
                # activity, so a skinny warm-up never trips it
                nc.tensor.matmul(
                    warm_ps[:, 0:O1], tscr[:, 0:128], tscr[:], start=True,
                    stop=True,
                )

            nc.vector.memset(t1k[:], 1.0)
            nc.vector.memset(t1r[:], 1.0)
            # identity for the activation transposes, on the idle GpSimd
            make_identity(nc, teye[:])

            # Everything runs in [batch, out] orientation: the per-batch
            # s1/s2 stats live as per-partition columns [B,1], so
            # alpha/beta apply as tensor_scalar operands with no
            # broadcast matmuls; biases fold into each z accumulation
            # via K=1 rank-1 matmuls of (ones row x bias row).

            def zgroup(tag, a_t, w_t, ics, ow, boff):
                """z_ps [B, ow] = a.T @ w + bias row, accumulated in the
                chunks' DMA arrival order."""
                z_ps = pp.tile([B, ow], f32, tag=tag)
                for j, ic in enumerate(ics):
                    nc.tensor.matmul(
                        z_ps[:],
                        a_t[:, ic * B : (ic + 1) * B],
                        w_t[:, ic * ow : (ic + 1) * ow],
                        start=(j == 0), stop=False,
                    )
                # bias rank-1 matmuls close the group; their bias-rows
                # tensor rides the scalar ring's first descriptors, so
                # these never extend the z tile's completion
                gw = min(ow, 128)
                for g in range(ow // gw):
                    nc.tensor.matmul(
                        z_ps[:, g * gw : (g + 1) * gw],
                        t1r[:],
                        tbr[0:1, boff + g * gw : boff + (g + 1) * gw],
                        start=False, stop=(g == ow // gw - 1),
                    )
                return z_ps

            def alphabeta(s1c, s2c, li):
                """alpha = k2*s1+1, beta = kb*s1+k0*s2 as [B,1] f32."""
                abc = ap.tile([B, 2], f32, tag=f"abc{li}")
                al, be = abc[:, 0:1], abc[:, 1:2]
                nc.vector.tensor_scalar(al, s1c, k2, 1.0, ALU.mult, ALU.add)
                nc.vector.tensor_scalar(be, s2c, k0, None, ALU.mult)
                nc.vector.scalar_tensor_tensor(be, s1c, kb, be, ALU.mult, ALU.add)
                return al, be

            def tail(z_ps, ow, al, be, a_next, s1n, s2n, li):
                """vj = relu(z); out = vj*alpha + k1*z + beta.  For inner
                layers, write the [B, ow] f16 result, transpose it back
                to [feature, batch] for the next layer's stationaries,
                and emit the next layer's s1/s2 sums for free via
                accum_out on the combine / square ops."""
                vj = ap.tile([B, ow], f32, tag=f"vj{li}")
                t_sb = ap.tile([B, ow], f32, tag=f"t{li}")
                nc.scalar.activation(vj[:], z_ps[:], AF.Relu)
                nc.vector.tensor_scalar(t_sb[:], z_ps[:], k1, be, ALU.mult, ALU.add)
                if a_next is None:
                    out_sb = ap.tile([B, ow], f32, tag="o3")
                    nc.vector.scalar_tensor_tensor(
                        out_sb[:], vj[:], al, t_sb[:], ALU.mult, ALU.add
                    )
                    return out_sb
                a_bo = ap.tile([B, ow], f16, tag=f"abo{li}")
                # asq must be f32: accum_out accumulates in the OUT
                # dtype, and s2 (a sum of ~512 positives) is garbage
                # under f16 sequential rounding
                asq = ap.tile([B, ow], f32, tag=f"asq{li}")
                nc.vector.scalar_tensor_tensor(
                    a_bo[:], vj[:], al, t_sb[:], ALU.mult, ALU.add,
                    accum_out=s1n,
                )
                nc.vector.scalar_tensor_tensor(
                    asq[:], a_bo[:], 1.0, a_bo[:], ALU.mult, ALU.mult,
                    accum_out=s2n,
                )
                aT_ps = pp.tile([128, (ow // 128) * B], f16, tag=f"aT{li}")
                for oc in range(ow // 128):
                    nc.tensor.transpose(
                        aT_ps[:, oc * B : (oc + 1) * B],
                        a_bo[:, oc * 128 : (oc + 1) * 128],
                        teye[:],
                    )
                nc.scalar.copy(a_next[:], aT_ps[:])

            # ---- layer 1: x stats via aT@ones column matmuls (they
            # keep the freshly warmed PE busy while fc1 streams in)
            xsq = ap.tile([128, 8 * B], f16, tag="xsq")
            nc.vector.tensor_tensor(xsq[:], tx[:], tx[:], ALU.mult)
            s1c1 = pp.tile([B, 1], f32, tag="s1c")
            s2c1 = pp.tile([B, 1], f32, tag="s2c")
            for c in range(8):
                nc.tensor.matmul(
                    s1c1[:], tx[:, c * B : (c + 1) * B], t1k[:],
                    start=(c == 0), stop=(c == 7),
                )
            for c in range(8):
                nc.tensor.matmul(
                    s2c1[:], xsq[:, c * B : (c + 1) * B], t1k[:],
                    start=(c == 0), stop=(c == 7),
                )
            al1, be1 = alphabeta(s1c1[:], s2c1[:], 1)

            s12 = ap.tile([B, 4], f32, tag="s12")  # s1/s2 cols, layers 2-3
            z1 = zgroup("z1", tx, tw1, [4, 5, 6, 7, 0, 1, 2, 3], O1, 0)
            a2 = ap.tile([128, 4 * B], f16, tag="a2")
            tail(z1, O1, al1, be1, a2, s12[:, 0:1], s12[:, 1:2], 1)

            # ---- layer 2
            al2, be2 = alphabeta(s12[:, 0:1], s12[:, 1:2], 2)
            z2 = zgroup("z2", a2[:], tw2, [2, 3, 0, 1], O2, O1)
            a3 = ap.tile([128, 4 * B], f16, tag="a3")
            tail(z2, O2, al2, be2, a3, s12[:, 2:3], s12[:, 3:4], 2)

            # ---- layer 3 (already [batch, out]; fc3 out-sharded)
            al3, be3 = alphabeta(s12[:, 2:3], s12[:, 3:4], 3)
            z3 = zgroup("z3", a3[:], tw3, [0, 1, 2, 3], O3L, O1 + O2)
            out_sb = tail(z3, O3L, al3, be3, None, None, None, 3)

            nc.sync.dma_start(out_d[:], out_sb[:], single_packet=True)

    nc.compile()
    return nc


def kernel(**inputs):
    from concourse.bass_utils import run_bass_kernel_spmd

    x = np.ascontiguousarray(np.asarray(inputs["x"], dtype=np.float32))
    fc1_w = np.asarray(inputs["fc1_w"], dtype=np.float32)
    fc1_b = np.asarray(inputs["fc1_b"], dtype=np.float32)
    fc2_w = np.asarray(inputs["fc2_w"], dtype=np.float32)
    fc2_b = np.asarray(inputs["fc2_b"], dtype=np.float32)
    fc3_w = np.asarray(inputs["fc3_w"], dtype=np.float32)
    fc3_b = np.asarray(inputs["fc3_b"], dtype=np.float32)
    c1w = np.asarray(inputs["conv1_w"], dtype=np.float32)
    c1b = np.asarray(inputs["conv1_b"], dtype=np.float32)
    c2w = np.asarray(inputs["conv2_w"], dtype=np.float32)
    c2b = np.asarray(inputs["conv2_b"], dtype=np.float32)
    bn = float(np.asarray(inputs["batch_num"]).astype(np.float64))

    scale = np.float32(RATE) / np.float32(bn)
    g = (c1w.T @ c2w[0]).astype(np.float32)  # [3]
    hb = np.float32(c1b @ c2w[0] + c2b[0])
    k0 = float(scale * g[0])
    k1 = float(scale * g[1])
    k2 = float(scale * g[2])
    kb = float(scale * hb)

    key = (k0, k1, k2, kb)
    if key not in _CACHE:
        _CACHE[key] = _build(*key)
    nc = _CACHE[key]

    def pack(m, n_c, width):  # [n_c*128, width] -> [128, n_c*width]
        return np.ascontiguousarray(
            m.reshape(n_c, 128, width).transpose(1, 0, 2).reshape(128, n_c * width)
        )

    xt_h = pack(x.T, 8, B).astype(np.float16)
    w1_h = pack(fc1_w.T, 8, O1).astype(np.float16)
    w2_base = pack(fc2_w.T, 4, O2).astype(np.float16)

    in_maps = []
    for c in range(N_CORES):
        br_h = np.zeros((1, BRW), dtype=np.float16)
        br_h[0, 0:O1] = fc1_b
        br_h[0, O1 : O1 + O2] = fc2_b
        br_h[0, O1 + O2 : O1 + O2 + O3L] = fc3_b[c * O3L : (c + 1) * O3L]
        w2c = np.zeros((128, W2W), dtype=np.float16)
        w2c[:, 0 : 4 * O2] = w2_base
        w2c[:, 4 * O2 :] = pack(
            fc3_w[c * O3L : (c + 1) * O3L].T, 4, O3L
        ).astype(np.float16)
        in_maps.append(dict(xt=xt_h, w1t=w1_h, w2t=w2c, brt=br_h))

    res = run_bass_kernel_spmd(nc, in_maps, list(range(N_CORES)))
    global LAST_RESULTS
    LAST_RESULTS = res
    return np.ascontiguousarray(
        np.concatenate([res.results[c]["out"] for c in range(N_CORES)], axis=1)
    ).astype(np.float32)


if __name__ == "__main__":
    rng = np.random.default_rng(0)

    def lin(fo, fi):
        bound = 1.0 / np.sqrt(fi)
        return (
            rng.uniform(-bound, bound, (fo, fi)).astype(np.float32),
            rng.uniform(-bound, bound, (fo,)).astype(np.float32),
        )

    fc1_w, fc1_b = lin(512, 1024)
    fc2_w, fc2_b = lin(512, 512)
    fc3_w, fc3_b = lin(256, 512)
    c1w, c1b = lin(8, 3)
    c2w, c2b = lin(1, 8)
    ins = dict(
        x=rng.standard_normal((32, 1024)).astype(np.float32),
        fc1_w=fc1_w, fc1_b=fc1_b, fc2_w=fc2_w, fc2_b=fc2_b,
        fc3_w=fc3_w, fc3_b=fc3_b,
        conv1_w=c1w, conv1_b=c1b, conv2_w=c2w, conv2_b=c2b,
        batch_num=10,
    )
    out = kernel(**ins)
    print("kernel out", out.shape, out.dtype, float(np.abs(out).max()))
